# revision 49
# baseline (speedup 1.0000x reference)
"""Trainium2 Bass kernel for Llama-style GQA attention (B=2, S=2048, HID=4096,
H=32 q-heads, KV=8 kv-heads, D=128), tensor-parallel over 8 NeuronCores.

Sharding: core c owns KV head c and its G=4 query heads (w_qkv row-sharded),
o_proj column-sharded; partial outputs ReduceScatter-summed per 512-token
group directly into the token-sharded external output; host concatenates.

Self-contained: hardcodes all shapes; only needs numpy/ml_dtypes + the
concourse (Bass/Tile) stack available in the environment.
"""

import os

import numpy as np
import ml_dtypes

import concourse.bass as bass
import concourse.mybir as mybir
from concourse.tile import TileContext
from concourse.bass_utils import run_bass_kernel_spmd

P = 128
NCORES = 8

# problem dims
B_FULL, S_FULL, HID_FULL = 2, 2048, 4096
H_FULL, KV_FULL, D_FULL = 32, 8, 128

BF16 = mybir.dt.bfloat16
F32 = mybir.dt.float32

LAST_RESULT = None  # BassKernelResults of the most recent run (for test harness)


def split_multi_waits(nc):
    """The walrus build in this container accepts at most ONE sync wait per
    instruction; Tile attaches one wait per producer proc. Hoist all-but-one
    wait onto standalone EventSemaphore instructions immediately before the
    instruction on the same engine (engine dispatch is in-order, so the
    semantics are identical)."""
    n = 0
    for f in nc.m.functions:
        for bb in f.blocks:
            out = []
            for inst in bb.instructions:
                si = inst.sync_info
                if si is not None and si.on_wait is not None and len(si.on_wait) > 1:
                    waits = list(si.on_wait)
                    for k, w in enumerate(waits[:-1]):
                        ev = mybir.InstEventSemaphore(
                            name=f"{inst.name}_wsplit{k}",
                            ins=[],
                            outs=[],
                            sync_info=mybir.SyncInfo(on_wait=[w], on_update=[]),
                        )
                        ev.engine = inst.engine
                        out.append(ev)
                        n += 1
                    si.on_wait.clear()
                    si.on_wait.append(waits[-1])
                out.append(inst)
            bb.instructions[:] = out
    return n


def build_nc(B, S, HID, G, D):
    """One SPMD program (identical on all cores; per-core data differs).

    Device inputs (per core c):
      hiddenT [HID, T]   bf16   hidden.reshape(T,HID).T        (replicated)
      wqkvT   [HID, F]   bf16   rows(c of w_qkv).T, F = (G+2)*D
      woT     [GD, HID]  bf16   w_o[:, c*GD:(c+1)*GD].T
      cosT    [D, T]     bf16   cos[b,s,:].T  (b-major tokens)
      ssinT   [D, T]     bf16   sin transposed, rows 0..D/2-1 negated
    Output:
      out     [T//8, HID] bf16: 8 chunks of 64 rows; chunk g holds global
      token rows [512*g + 64*c, 512*g + 64*(c+1)) of the summed output.
    """
    T = B * S
    F = (G + 2) * D            # per-core qkv features (q heads | k | v)
    NF = F // P                # feature chunks (6)
    KH = HID // P              # hidden contraction chunks (32)
    GD = G * D                 # per-core attn-out features (512)

    QS = 512                   # token group = q supertile = RS chunk
    NG = T // QS               # groups (8)
    NQS = S // QS              # q supertiles per batch
    NKB = S // P               # k blocks per batch
    KB_PER_QS = QS // P        # k blocks spanned by one q supertile (4)

    HB = 512                   # o_proj hid tile
    NHB = HID // HB
    NFO = GD // P              # o_proj contraction chunks (4)
    RPC = QS // NCORES         # output rows per chunk per core (64)

    SCALE = 1.0 / float(np.sqrt(D))

    nc = bass.Bass()
    hiddenT = nc.dram_tensor("hiddenT", [HID, T], BF16, kind="ExternalInput")
    wqkvT = nc.dram_tensor("wqkvT", [HID, F], BF16, kind="ExternalInput")
    woT = nc.dram_tensor("woT", [GD, HID], BF16, kind="ExternalInput")
    cosT = nc.dram_tensor("cosT", [D, T], BF16, kind="ExternalInput")
    ssinT = nc.dram_tensor("ssinT", [D, T], BF16, kind="ExternalInput")
    out_ext = nc.dram_tensor("out", [T // NCORES, HID], BF16,
                             kind="ExternalOutput")

    with TileContext(nc) as tc:
        with (
            tc.tile_pool(name="big", bufs=1) as big,          # resident tensors
            tc.tile_pool(name="htile", bufs=2) as htile,      # hiddenT stream
            tc.tile_pool(name="qstream", bufs=2) as qstream,  # q per group
            tc.tile_pool(name="attnring", bufs=3) as attnring,
            tc.tile_pool(name="wostream", bufs=2) as wostream,
            tc.tile_pool(name="small", bufs=1) as small,      # masks/identity
            tc.tile_pool(name="work", bufs=4) as work,        # copies in flight
            tc.tile_pool(name="ropep", bufs=2) as ropep,
            tc.tile_pool(name="ps_acc", bufs=4, space="PSUM") as ps_acc,
            tc.tile_pool(name="ps_st", bufs=2, space="PSUM") as ps_st,
            tc.tile_pool(name="ps_mm", bufs=2, space="PSUM") as ps_mm,
            tc.tile_pool(name="dram", bufs=1, space="DRAM") as dram,
        ):
            # ---------------- resident loads ----------------
            # wqkv loads in 6 f-major pieces (one per feature chunk) so the
            # first f-chain only needs 1MB before PE can start
            w_sb = big.tile([P, KH, F], BF16, tag="w_sb")
            wqkvT_r = wqkvT.rearrange("(kh p) f -> p kh f", p=P)

            def emit_w_piece(f):
                nc.sync.dma_start(out=w_sb[:, :, f * P:(f + 1) * P],
                                  in_=wqkvT_r[:, :, f * P:(f + 1) * P])

            # first piece in kh-halves so the very first matmul chain only
            # waits on 0.5MB of weights
            nc.sync.dma_start(out=w_sb[:, 0:KH // 2, 0:P],
                              in_=wqkvT_r[:, 0:KH // 2, 0:P])
            nc.sync.dma_start(out=w_sb[:, KH // 2:KH, 0:P],
                              in_=wqkvT_r[:, KH // 2:KH, 0:P])
            emit_w_piece(1)
            w_rest = list(range(2, NF))

            # K for both batches, resident; V in [tok, d] layout + ones col
            kT_sb = big.tile([P, B, S], BF16, tag="kT_sb")
            v_sb = big.tile([P, B, NKB, D + 4], BF16, tag="v_sb")
            nc.vector.memset(v_sb[:, :, :, D:D + 1], 1.0)

            ident = small.tile([P, P], BF16, tag="ident")
            nc.gpsimd.memset(ident[:], 0.0)
            nc.gpsimd.affine_select(
                out=ident[:], in_=ident[:],
                compare_op=mybir.AluOpType.not_equal, fill=1.0,
                base=0, pattern=[[-1, P]], channel_multiplier=1,
            )

            # causal mask for the diagonal 128x128 block: 1 iff j >= i
            mask128 = small.tile([P, P], BF16, tag="mask128")
            nc.gpsimd.memset(mask128[:], 1.0)
            nc.gpsimd.affine_select(
                out=mask128[:], in_=mask128[:],
                compare_op=mybir.AluOpType.is_ge, fill=0.0,
                base=0, pattern=[[1, P]], channel_multiplier=-1,
            )

            # woT resident; pieces emitted during group 0's projection
            wo_sb_res = big.tile([P, NFO, HID], BF16, tag="wo_sb_res")
            woT_rr = woT.rearrange("(f p) h -> p f h", p=P)
            WOP = HID // 4

            def emit_wo_piece(i):
                nc.sync.dma_start(
                    out=wo_sb_res[:, :, i * WOP:(i + 1) * WOP],
                    in_=woT_rr[:, :, i * WOP:(i + 1) * WOP])

            wo_rest = list(range(4))

            hiddenT_r = hiddenT.rearrange("(kh p) t -> p kh t", p=P)

            # ---------------- phase 1: QKV + RoPE + V transpose -----------
            def emit_p1_head(g):
                """Allocate tiles and issue the DMA loads for one 512-token
                group (so they start early); compute is in emit_p1_chains."""
                t0 = g * QS
                b, qs = t0 // S, (t0 % S) // QS
                h_sb = htile.tile([P, KH, QS], BF16, tag="h_sb", name="h_sb")
                step = KH // 8 if g == 0 else KH // 4   # finer at startup
                for k4 in range(0, KH, step):
                    nc.sync.dma_start(
                        out=h_sb[:, k4:k4 + step, :],
                        in_=hiddenT_r[:, k4:k4 + step, t0:t0 + QS])
                cs_sb = htile.tile([P, QS], BF16, tag="cs_sb", name="cs_sb")
                nc.sync.dma_start(out=cs_sb[:], in_=cosT[:, t0:t0 + QS])
                ss_sb = htile.tile([P, QS], BF16, tag="ss_sb", name="ss_sb")
                nc.sync.dma_start(out=ss_sb[:], in_=ssinT[:, t0:t0 + QS])
                q_sb = qstream.tile([P, G, QS], BF16, tag="q_sb", name="q_sb")
                return g, b, qs, h_sb, cs_sb, ss_sb, q_sb

            def emit_p1_chains(hd):
                """Generator: one yield per QKV f-chain, then RoPE and the
                V transpose on the final advance. Interleavable into the
                previous group's attention as PE filler."""
                g, b, qs, h_sb, cs_sb, ss_sb, q_sb = hd
                vtmp = ropep.tile([P, QS], BF16, tag="vtmp", name="vtmp",
                                  bufs=1)
                for f in range(NF):
                    if g == 0 and w_rest:
                        emit_w_piece(w_rest.pop(0))
                    if g == 0 and f >= 2 and wo_rest:
                        emit_wo_piece(wo_rest.pop(0))
                    ps = ps_mm.tile([P, QS], F32, tag="mm", name="ps")
                    for k in range(KH):
                        nc.tensor.matmul(ps[:], w_sb[:, k, f * P:(f + 1) * P],
                                         h_sb[:, k, :],
                                         start=(k == 0), stop=(k == KH - 1))
                    if f < G:
                        dst = q_sb[:, f, :]
                    elif f == G:
                        dst = kT_sb[:, b, qs * QS:(qs + 1) * QS]
                    else:
                        dst = vtmp[:]
                    nc.vector.tensor_copy(dst, ps[:])
                    yield

                # RoPE on q heads and k, in place
                for f in range(G + 1):
                    x = (q_sb[:, f, :] if f < G
                         else kT_sb[:, b, qs * QS:(qs + 1) * QS])
                    r = ropep.tile([P, QS], BF16, tag="rope_r", name="r")
                    nc.vector.tensor_copy(r[0:D // 2, :], x[D // 2:D, :])
                    nc.vector.tensor_copy(r[D // 2:D, :], x[0:D // 2, :])
                    nc.vector.tensor_mul(x, x, cs_sb[:])
                    nc.vector.tensor_mul(r[:], r[:], ss_sb[:])
                    nc.vector.tensor_add(x, x, r[:])

                # V transpose into [tok, d] (PE transpose per 128-token block)
                for j in range(KB_PER_QS):
                    kb = qs * KB_PER_QS + j
                    tp = ps_st.tile([P, P], BF16, tag="st", name="tp")
                    nc.tensor.transpose(tp[:], vtmp[:, j * P:(j + 1) * P],
                                        ident[:])
                    nc.scalar.copy(v_sb[:, b, kb, 0:D], tp[:])

            # ---------------- phase 2: flash attention --------------------
            def emit_attention(g, q_sb, att, q0, q1, filler=None):
                """Flash attention for q columns [q0, q1) of one group
                (batch b, supertile qs). `filler` is a generator emitting
                independent PE work (pending o_proj chains); one chunk is
                pulled every few pipeline steps so the ACT exp latency
                (~450ns) hides behind guaranteed-ready matmuls instead of
                stalling PV."""
                t0 = g * QS
                b, qs = t0 // S, (t0 % S) // QS
                kT = kT_sb[:, b, 0:S]
                NJ = (q1 - q0) // P          # q blocks this pass
                kq0 = qs * KB_PER_QS + q0 // P   # first diagonal k block
                nkb = kq0 + NJ               # causal: kb in [0, nkb)

                def emit_scores(h, kb):
                    # diagonal superblocks only need q columns >= r*P
                    r = kb - kq0
                    w0 = max(r, 0) * P   # first valid q column (relative)
                    W = (q1 - q0) - w0
                    sT = ps_st.tile([P, QS], F32, tag="st",
                                    name="sT")[:, 0:W]
                    nc.tensor.matmul(sT, kT[:, kb * P:(kb + 1) * P],
                                     q_sb[:, h, q0 + w0:q1],
                                     start=True, stop=True)
                    pT = work.tile([P, QS], BF16, tag="pT",
                                   name="pT", bufs=4)[:, 0:W]
                    nc.scalar.activation(
                        pT, sT, mybir.ActivationFunctionType.Exp,
                        scale=SCALE)
                    if r >= 0:
                        # only the j == r sub-block straddles the causal
                        # diagonal; later sub-blocks are fully valid
                        nc.vector.tensor_mul(
                            pT[:, 0:P], pT[:, 0:P], mask128[:])
                    return h, kb, pT, w0

                def finalize_head(h, acc):
                    for j in range(NJ):
                        recip = work.tile([P, 1], F32, tag="recip",
                                          name="recip", bufs=2)
                        nc.vector.reciprocal(recip[:], acc[j][:, D:D + 1])
                        o_sb = work.tile([P, D], BF16, tag="o_sb",
                                         name="o_sb", bufs=2)
                        nc.vector.tensor_scalar_mul(
                            o_sb[:], acc[j][:, 0:D], recip[:])
                        tp = ps_st.tile([P, P], BF16, tag="st", name="tp")
                        nc.tensor.transpose(tp[:], o_sb[:], ident[:])
                        nc.vector.tensor_copy(
                            att[:, h, q0 + j * P:q0 + (j + 1) * P], tp[:])

                # flat (h, kb) pipeline with one-block scores lookahead that
                # crosses head boundaries, so PE never idles on head tails
                items = [(h, kb) for h in range(G) for kb in range(nkb)]
                stride = max(1, len(items) // 12)
                pend = [emit_scores(*items[0])]
                acc = None
                for idx in range(len(items)):
                    if idx + 1 < len(items):
                        pend.append(emit_scores(*items[idx + 1]))
                    if filler is not None and idx % stride == 0:
                        next(filler, None)
                    h, kb, pT, w0 = pend.pop(0)
                    if kb == 0:
                        acc = [ps_acc.tile([P, D + 4], F32, tag="acc",
                                           name=f"acc{j}")
                               for j in range(NJ)]
                    for j in range(w0 // P, NJ):
                        if kb > kq0 + j:
                            continue  # fully masked block
                        nc.tensor.matmul(
                            acc[j][:, 0:D + 1],
                            pT[:, j * P - w0:(j + 1) * P - w0],
                            v_sb[:, b, kb, 0:D + 1],
                            start=(kb == 0),
                            stop=(kb == kq0 + j))
                    if kb == nkb - 1:
                        finalize_head(h, acc)

            # ---------------- phase 3: o_proj + per-group ReduceScatter ---
            partials = [dram.tile([QS, HID], BF16, tag=f"partial{g}",
                                  name=f"partial{g}")
                        for g in range(NG)]
            rs_outs = [dram.tile([RPC, HID], BF16, tag=f"rs_out{g}",
                                 name=f"rs_out{g}")
                       for g in range(NG)]

            def emit_rs(g, lo, hi):
                """ReduceScatter token rows [lo,hi) of group g's partial into
                the matching out_ext rows."""
                n = (hi - lo) // NCORES
                r0 = lo // NCORES
                o0 = g * RPC + r0
                nc.gpsimd.collective_compute(
                    "ReduceScatter",
                    mybir.AluOpType.add,
                    replica_groups=[list(range(NCORES))],
                    ins=[partials[g][lo:hi, :]],
                    outs=[rs_outs[g][r0:r0 + n, :]],
                )
                nc.gpsimd.dma_start(
                    out=out_ext[o0:o0 + n, :],
                    in_=rs_outs[g][r0:r0 + n, :])

            def emit_p3(g, att, tb_lo, tb_hi):
                """Generator: one yield per po half-block (16 matmuls), so
                the o_proj for group g can interleave into later attention.
                Finishes with the ReduceScatter over the covered rows."""
                partial_r = partials[g].rearrange("(tb p) h -> p tb h", p=P)
                HBH = NHB // 2
                for tb in range(tb_lo, tb_hi):
                    for half in range(2):
                        po = wostream.tile([P, HBH * HB], BF16, tag="po",
                                           name="po", bufs=3)
                        for hh in range(HBH):
                            hb = half * HBH + hh
                            ps = ps_mm.tile([P, HB], F32, tag="mm", name="ps")
                            for fb in range(NFO):
                                nc.tensor.matmul(
                                    ps[:],
                                    att[:, fb, tb * P:(tb + 1) * P],
                                    wo_sb_res[:, fb, hb * HB:(hb + 1) * HB],
                                    start=(fb == 0),
                                    stop=(fb == NFO - 1))
                            nc.vector.tensor_copy(
                                po[:, hh * HB:(hh + 1) * HB], ps[:])
                        nc.scalar.dma_start(
                            out=partial_r[:, tb,
                                          half * HBH * HB:(half + 1) * HBH * HB],
                            in_=po[:])
                        yield
                emit_rs(g, tb_lo * P, tb_hi * P)

            # ---------------- driver --------------------------------------
            def chain_gens(*gens):
                for gen in gens:
                    if gen is not None:
                        yield from gen

            # bootstrap group 0's projection inline
            hd = emit_p1_head(0)
            for _ in emit_p1_chains(hd):
                pass
            prev_p3 = None
            for g in range(NG):
                q_sb = hd[6]
                # next group's loads start now; its f-chains ride the filler
                nxt = None
                if g + 1 < NG:
                    hd = emit_p1_head(g + 1)
                    nxt = emit_p1_chains(hd)
                att = attnring.tile([P, G, QS], BF16, tag="att", name="att")
                if g < NG - 1:
                    filler = chain_gens(prev_p3, nxt)
                    emit_attention(g, q_sb, att, 0, QS, filler=filler)
                    for _ in filler:    # drain o_proj blocks + qkv chains
                        pass
                    prev_p3 = emit_p3(g, att, 0, KB_PER_QS)
                else:
                    # last group: two q-halves so the first half's o_proj +
                    # RS overlap the second half's attention
                    emit_attention(g, q_sb, att, 0, QS // 2, filler=prev_p3)
                    if prev_p3 is not None:
                        for _ in prev_p3:
                            pass
                    pa = emit_p3(g, att, 0, KB_PER_QS // 2)
                    emit_attention(g, q_sb, att, QS // 2, QS, filler=pa)
                    for _ in pa:
                        pass
                    prev_p3 = emit_p3(g, att, KB_PER_QS // 2, KB_PER_QS)
            for _ in prev_p3:
                pass

    split_multi_waits(nc)
    return nc


_NC_CACHE = {}


def _get_nc(key):
    if key not in _NC_CACHE:
        _NC_CACHE[key] = build_nc(*key)
    return _NC_CACHE[key]


def run(hidden_states, w_qkv, w_o, cos, sin, B, S, HID, H, KV, D,
        trace=False):
    G = H // KV
    T = B * S
    GD = G * D
    QS = 512
    RPC = QS // NCORES
    assert KV == NCORES
    nc = _get_nc((B, S, HID, G, D))

    bf = ml_dtypes.bfloat16
    hiddenT = np.ascontiguousarray(
        hidden_states.reshape(T, HID).T).astype(bf)
    cosT = np.ascontiguousarray(
        cos.transpose(2, 0, 1).reshape(D, T)).astype(bf)
    sinT = np.ascontiguousarray(
        sin.transpose(2, 0, 1).reshape(D, T)).astype(np.float32)
    ssinT = sinT.copy()
    ssinT[:D // 2] *= -1.0
    ssinT = ssinT.astype(bf)

    in_maps = []
    for c in range(NCORES):
        qrows = w_qkv[c * GD:(c + 1) * GD]               # G query heads
        krows = w_qkv[H * D + c * D: H * D + (c + 1) * D]
        vrows = w_qkv[(H + KV) * D + c * D: (H + KV) * D + (c + 1) * D]
        w_c = np.concatenate([qrows, krows, vrows], axis=0)   # [F, HID]
        wqkvT = np.ascontiguousarray(w_c.T).astype(bf)
        woT = np.ascontiguousarray(w_o[:, c * GD:(c + 1) * GD].T).astype(bf)
        in_maps.append({
            "hiddenT": hiddenT, "wqkvT": wqkvT, "woT": woT,
            "cosT": cosT, "ssinT": ssinT,
        })

    res = run_bass_kernel_spmd(nc, in_maps, core_ids=list(range(NCORES)),
                               trace=trace)
    global LAST_RESULT
    LAST_RESULT = res

    # chunk list must match emit_p3/emit_rs: full groups of 512, the last
    # group split into two 256-token halves
    chunks = [(g * QS, QS) for g in range(T // QS - 1)]
    chunks += [(T - QS, QS // 2), (T - QS // 2, QS // 2)]
    full = np.empty((T, HID), dtype=np.float32)
    for c in range(NCORES):
        shard = res.results[c]["out"].astype(np.float32)  # [T//8, HID]
        o = 0
        for t0, ln in chunks:
            rpc = ln // NCORES
            full[t0 + c * rpc: t0 + (c + 1) * rpc] = shard[o:o + rpc]
            o += rpc
    return full.reshape(B, S, HID)


def kernel(hidden_states, w_qkv, w_o, cos, sin):
    return run(np.asarray(hidden_states), np.asarray(w_qkv), np.asarray(w_o),
               np.asarray(cos), np.asarray(sin),
               B_FULL, S_FULL, HID_FULL, H_FULL, KV_FULL, D_FULL,
               trace=bool(int(os.environ.get("KERNEL_TRACE", "0"))))


# revision 50
# speedup vs baseline: 1.0093x; 1.0093x over previous
"""Trainium2 Bass kernel for Llama-style GQA attention (B=2, S=2048, HID=4096,
H=32 q-heads, KV=8 kv-heads, D=128), tensor-parallel over 8 NeuronCores.

Sharding: core c owns KV head c and its G=4 query heads (w_qkv row-sharded),
o_proj column-sharded; partial outputs ReduceScatter-summed per 512-token
group directly into the token-sharded external output; host concatenates.

Self-contained: hardcodes all shapes; only needs numpy/ml_dtypes + the
concourse (Bass/Tile) stack available in the environment.
"""

import os

import numpy as np
import ml_dtypes

import concourse.bass as bass
import concourse.mybir as mybir
from concourse.tile import TileContext
from concourse.bass_utils import run_bass_kernel_spmd

P = 128
NCORES = 8

# problem dims
B_FULL, S_FULL, HID_FULL = 2, 2048, 4096
H_FULL, KV_FULL, D_FULL = 32, 8, 128

BF16 = mybir.dt.bfloat16
F32 = mybir.dt.float32

LAST_RESULT = None  # BassKernelResults of the most recent run (for test harness)


def split_multi_waits(nc):
    """The walrus build in this container accepts at most ONE sync wait per
    instruction; Tile attaches one wait per producer proc. Hoist all-but-one
    wait onto standalone EventSemaphore instructions immediately before the
    instruction on the same engine (engine dispatch is in-order, so the
    semantics are identical)."""
    n = 0
    for f in nc.m.functions:
        for bb in f.blocks:
            out = []
            for inst in bb.instructions:
                si = inst.sync_info
                if si is not None and si.on_wait is not None and len(si.on_wait) > 1:
                    waits = list(si.on_wait)
                    for k, w in enumerate(waits[:-1]):
                        ev = mybir.InstEventSemaphore(
                            name=f"{inst.name}_wsplit{k}",
                            ins=[],
                            outs=[],
                            sync_info=mybir.SyncInfo(on_wait=[w], on_update=[]),
                        )
                        ev.engine = inst.engine
                        out.append(ev)
                        n += 1
                    si.on_wait.clear()
                    si.on_wait.append(waits[-1])
                out.append(inst)
            bb.instructions[:] = out
    return n


def build_nc(B, S, HID, G, D):
    """One SPMD program (identical on all cores; per-core data differs).

    Device inputs (per core c):
      hiddenT [HID, T]   bf16   hidden.reshape(T,HID).T        (replicated)
      wqkvT   [HID, F]   bf16   rows(c of w_qkv).T, F = (G+2)*D
      woT     [GD, HID]  bf16   w_o[:, c*GD:(c+1)*GD].T
      cosT    [D, T]     bf16   cos[b,s,:].T  (b-major tokens)
      ssinT   [D, T]     bf16   sin transposed, rows 0..D/2-1 negated
    Output:
      out     [T//8, HID] bf16: 8 chunks of 64 rows; chunk g holds global
      token rows [512*g + 64*c, 512*g + 64*(c+1)) of the summed output.
    """
    T = B * S
    F = (G + 2) * D            # per-core qkv features (q heads | k | v)
    NF = F // P                # feature chunks (6)
    KH = HID // P              # hidden contraction chunks (32)
    GD = G * D                 # per-core attn-out features (512)

    QS = 512                   # token group = q supertile = RS chunk
    NG = T // QS               # groups (8)
    NQS = S // QS              # q supertiles per batch
    NKB = S // P               # k blocks per batch
    KB_PER_QS = QS // P        # k blocks spanned by one q supertile (4)

    HB = 512                   # o_proj hid tile
    NHB = HID // HB
    NFO = GD // P              # o_proj contraction chunks (4)
    RPC = QS // NCORES         # output rows per chunk per core (64)

    SCALE = 1.0 / float(np.sqrt(D))

    nc = bass.Bass()
    hiddenT = nc.dram_tensor("hiddenT", [HID, T], BF16, kind="ExternalInput")
    wqkvT = nc.dram_tensor("wqkvT", [HID, F], BF16, kind="ExternalInput")
    woT = nc.dram_tensor("woT", [GD, HID], BF16, kind="ExternalInput")
    cosT = nc.dram_tensor("cosT", [D, T], BF16, kind="ExternalInput")
    ssinT = nc.dram_tensor("ssinT", [D, T], BF16, kind="ExternalInput")
    out_ext = nc.dram_tensor("out", [T // NCORES, HID], BF16,
                             kind="ExternalOutput")

    with TileContext(nc) as tc:
        with (
            tc.tile_pool(name="big", bufs=1) as big,          # resident tensors
            tc.tile_pool(name="htile", bufs=2) as htile,      # hiddenT stream
            tc.tile_pool(name="qstream", bufs=2) as qstream,  # q per group
            tc.tile_pool(name="attnring", bufs=3) as attnring,
            tc.tile_pool(name="wostream", bufs=2) as wostream,
            tc.tile_pool(name="small", bufs=1) as small,      # masks/identity
            tc.tile_pool(name="work", bufs=4) as work,        # copies in flight
            tc.tile_pool(name="ropep", bufs=2) as ropep,
            tc.tile_pool(name="ps_acc", bufs=4, space="PSUM") as ps_acc,
            tc.tile_pool(name="ps_st", bufs=2, space="PSUM") as ps_st,
            tc.tile_pool(name="ps_mm", bufs=2, space="PSUM") as ps_mm,
            tc.tile_pool(name="dram", bufs=1, space="DRAM") as dram,
        ):
            # ---------------- resident loads ----------------
            # wqkv loads in 6 f-major pieces (one per feature chunk) so the
            # first f-chain only needs 1MB before PE can start
            w_sb = big.tile([P, KH, F], BF16, tag="w_sb")
            wqkvT_r = wqkvT.rearrange("(kh p) f -> p kh f", p=P)

            def emit_w_piece(f):
                nc.sync.dma_start(out=w_sb[:, :, f * P:(f + 1) * P],
                                  in_=wqkvT_r[:, :, f * P:(f + 1) * P])

            # first piece in kh-halves so the very first matmul chain only
            # waits on 0.5MB of weights
            nc.sync.dma_start(out=w_sb[:, 0:KH // 2, 0:P],
                              in_=wqkvT_r[:, 0:KH // 2, 0:P])
            nc.sync.dma_start(out=w_sb[:, KH // 2:KH, 0:P],
                              in_=wqkvT_r[:, KH // 2:KH, 0:P])
            emit_w_piece(1)
            w_rest = list(range(2, NF))

            # K for both batches, resident; V in [tok, d] layout + ones col
            kT_sb = big.tile([P, B, S], BF16, tag="kT_sb")
            v_sb = big.tile([P, B, NKB, D + 4], BF16, tag="v_sb")
            nc.vector.memset(v_sb[:, :, :, D:D + 1], 1.0)

            ident = small.tile([P, P], BF16, tag="ident")
            nc.gpsimd.memset(ident[:], 0.0)
            nc.gpsimd.affine_select(
                out=ident[:], in_=ident[:],
                compare_op=mybir.AluOpType.not_equal, fill=1.0,
                base=0, pattern=[[-1, P]], channel_multiplier=1,
            )

            # causal mask for the diagonal 128x128 block: 1 iff j >= i
            mask128 = small.tile([P, P], BF16, tag="mask128")
            nc.gpsimd.memset(mask128[:], 1.0)
            nc.gpsimd.affine_select(
                out=mask128[:], in_=mask128[:],
                compare_op=mybir.AluOpType.is_ge, fill=0.0,
                base=0, pattern=[[1, P]], channel_multiplier=-1,
            )

            # woT resident; pieces emitted during group 0's projection
            wo_sb_res = big.tile([P, NFO, HID], BF16, tag="wo_sb_res")
            woT_rr = woT.rearrange("(f p) h -> p f h", p=P)
            WOP = HID // 4

            def emit_wo_piece(i):
                nc.sync.dma_start(
                    out=wo_sb_res[:, :, i * WOP:(i + 1) * WOP],
                    in_=woT_rr[:, :, i * WOP:(i + 1) * WOP])

            wo_rest = list(range(4))

            hiddenT_r = hiddenT.rearrange("(kh p) t -> p kh t", p=P)

            # ---------------- phase 1: QKV + RoPE + V transpose -----------
            def emit_p1_head(g):
                """Allocate tiles and issue the DMA loads for one 512-token
                group (so they start early); compute is in emit_p1_chains."""
                t0 = g * QS
                b, qs = t0 // S, (t0 % S) // QS
                h_sb = htile.tile([P, KH, QS], BF16, tag="h_sb", name="h_sb")
                step = KH // 8 if g == 0 else KH // 4   # finer at startup
                for k4 in range(0, KH, step):
                    nc.sync.dma_start(
                        out=h_sb[:, k4:k4 + step, :],
                        in_=hiddenT_r[:, k4:k4 + step, t0:t0 + QS])
                cs_sb = htile.tile([P, QS], BF16, tag="cs_sb", name="cs_sb")
                nc.sync.dma_start(out=cs_sb[:], in_=cosT[:, t0:t0 + QS])
                ss_sb = htile.tile([P, QS], BF16, tag="ss_sb", name="ss_sb")
                nc.sync.dma_start(out=ss_sb[:], in_=ssinT[:, t0:t0 + QS])
                q_sb = qstream.tile([P, G, QS], BF16, tag="q_sb", name="q_sb")
                return g, b, qs, h_sb, cs_sb, ss_sb, q_sb

            def emit_p1_chains(hd):
                """Generator: one yield per QKV f-chain, then RoPE and the
                V transpose on the final advance. Interleavable into the
                previous group's attention as PE filler."""
                g, b, qs, h_sb, cs_sb, ss_sb, q_sb = hd
                vtmp = ropep.tile([P, QS], BF16, tag="vtmp", name="vtmp",
                                  bufs=1)
                for f in range(NF):
                    if g == 0 and w_rest:
                        emit_w_piece(w_rest.pop(0))
                    if g == 0 and f >= 2 and wo_rest:
                        emit_wo_piece(wo_rest.pop(0))
                    ps = ps_mm.tile([P, QS], F32, tag="mm", name="ps")
                    for k in range(KH):
                        nc.tensor.matmul(ps[:], w_sb[:, k, f * P:(f + 1) * P],
                                         h_sb[:, k, :],
                                         start=(k == 0), stop=(k == KH - 1))
                    if f < G:
                        dst = q_sb[:, f, :]
                    elif f == G:
                        dst = kT_sb[:, b, qs * QS:(qs + 1) * QS]
                    else:
                        dst = vtmp[:]
                    nc.vector.tensor_copy(dst, ps[:])
                    yield

                # RoPE on q heads and k, in place
                for f in range(G + 1):
                    x = (q_sb[:, f, :] if f < G
                         else kT_sb[:, b, qs * QS:(qs + 1) * QS])
                    r = ropep.tile([P, QS], BF16, tag="rope_r", name="r")
                    nc.vector.tensor_copy(r[0:D // 2, :], x[D // 2:D, :])
                    nc.vector.tensor_copy(r[D // 2:D, :], x[0:D // 2, :])
                    nc.vector.tensor_mul(x, x, cs_sb[:])
                    nc.vector.tensor_mul(r[:], r[:], ss_sb[:])
                    nc.vector.tensor_add(x, x, r[:])

                # V transpose into [tok, d] (PE transpose per 128-token block)
                for j in range(KB_PER_QS):
                    kb = qs * KB_PER_QS + j
                    tp = ps_st.tile([P, P], BF16, tag="st", name="tp")
                    nc.tensor.transpose(tp[:], vtmp[:, j * P:(j + 1) * P],
                                        ident[:])
                    nc.scalar.copy(v_sb[:, b, kb, 0:D], tp[:])

            # ---------------- phase 2: flash attention --------------------
            def emit_attention(g, q_sb, att, q0, q1, filler=None):
                """Flash attention for q columns [q0, q1) of one group
                (batch b, supertile qs). `filler` is a generator emitting
                independent PE work (pending o_proj chains); one chunk is
                pulled every few pipeline steps so the ACT exp latency
                (~450ns) hides behind guaranteed-ready matmuls instead of
                stalling PV."""
                t0 = g * QS
                b, qs = t0 // S, (t0 % S) // QS
                kT = kT_sb[:, b, 0:S]
                NJ = (q1 - q0) // P          # q blocks this pass
                kq0 = qs * KB_PER_QS + q0 // P   # first diagonal k block
                nkb = kq0 + NJ               # causal: kb in [0, nkb)

                def emit_scores(h, kb):
                    # diagonal superblocks only need q columns >= r*P
                    r = kb - kq0
                    w0 = max(r, 0) * P   # first valid q column (relative)
                    W = (q1 - q0) - w0
                    sT = ps_st.tile([P, QS], F32, tag="st",
                                    name="sT")[:, 0:W]
                    nc.tensor.matmul(sT, kT[:, kb * P:(kb + 1) * P],
                                     q_sb[:, h, q0 + w0:q1],
                                     start=True, stop=True)
                    pT = work.tile([P, QS], BF16, tag="pT",
                                   name="pT", bufs=4)[:, 0:W]
                    nc.scalar.activation(
                        pT, sT, mybir.ActivationFunctionType.Exp,
                        scale=SCALE)
                    if r >= 0:
                        # only the j == r sub-block straddles the causal
                        # diagonal; later sub-blocks are fully valid
                        nc.vector.tensor_mul(
                            pT[:, 0:P], pT[:, 0:P], mask128[:])
                    return h, kb, pT, w0

                def finalize_head(h, acc):
                    for j in range(NJ):
                        recip = work.tile([P, 1], F32, tag="recip",
                                          name="recip", bufs=2)
                        nc.vector.reciprocal(recip[:], acc[j][:, D:D + 1])
                        o_sb = work.tile([P, D], BF16, tag="o_sb",
                                         name="o_sb", bufs=2)
                        nc.vector.tensor_scalar_mul(
                            o_sb[:], acc[j][:, 0:D], recip[:])
                        tp = ps_st.tile([P, P], BF16, tag="st", name="tp")
                        nc.tensor.transpose(tp[:], o_sb[:], ident[:])
                        nc.vector.tensor_copy(
                            att[:, h, q0 + j * P:q0 + (j + 1) * P], tp[:])

                # flat (h, kb) pipeline with one-block scores lookahead that
                # crosses head boundaries, so PE never idles on head tails
                items = [(h, kb) for h in range(G) for kb in range(nkb)]
                stride = max(1, len(items) // 12)
                pend = [emit_scores(*items[0])]
                acc = None
                for idx in range(len(items)):
                    if idx + 1 < len(items):
                        pend.append(emit_scores(*items[idx + 1]))
                    if filler is not None and idx % stride == 0:
                        next(filler, None)
                    h, kb, pT, w0 = pend.pop(0)
                    if kb == 0:
                        acc = [ps_acc.tile([P, D + 4], F32, tag="acc",
                                           name=f"acc{j}")
                               for j in range(NJ)]
                    for j in range(w0 // P, NJ):
                        if kb > kq0 + j:
                            continue  # fully masked block
                        nc.tensor.matmul(
                            acc[j][:, 0:D + 1],
                            pT[:, j * P - w0:(j + 1) * P - w0],
                            v_sb[:, b, kb, 0:D + 1],
                            start=(kb == 0),
                            stop=(kb == kq0 + j))
                    if kb == nkb - 1:
                        finalize_head(h, acc)

            # ---------------- phase 3: o_proj + per-group ReduceScatter ---
            partials = [dram.tile([QS, HID], BF16, tag=f"partial{g}",
                                  name=f"partial{g}")
                        for g in range(NG)]
            rs_outs = [dram.tile([RPC, HID], BF16, tag=f"rs_out{g}",
                                 name=f"rs_out{g}")
                       for g in range(NG)]

            def emit_rs(g, lo, hi):
                """ReduceScatter token rows [lo,hi) of group g's partial into
                the matching out_ext rows."""
                n = (hi - lo) // NCORES
                r0 = lo // NCORES
                o0 = g * RPC + r0
                nc.gpsimd.collective_compute(
                    "ReduceScatter",
                    mybir.AluOpType.add,
                    replica_groups=[list(range(NCORES))],
                    ins=[partials[g][lo:hi, :]],
                    outs=[rs_outs[g][r0:r0 + n, :]],
                )
                nc.gpsimd.dma_start(
                    out=out_ext[o0:o0 + n, :],
                    in_=rs_outs[g][r0:r0 + n, :])

            def emit_p3(g, att, tb_lo, tb_hi):
                """Generator: one yield per po half-block (16 matmuls), so
                the o_proj for group g can interleave into later attention.
                Finishes with the ReduceScatter over the covered rows."""
                partial_r = partials[g].rearrange("(tb p) h -> p tb h", p=P)
                HBH = NHB // 2
                for tb in range(tb_lo, tb_hi):
                    for half in range(2):
                        po = wostream.tile([P, HBH * HB], BF16, tag="po",
                                           name="po", bufs=3)
                        for hh in range(HBH):
                            hb = half * HBH + hh
                            ps = ps_mm.tile([P, HB], F32, tag="mm", name="ps")
                            for fb in range(NFO):
                                nc.tensor.matmul(
                                    ps[:],
                                    att[:, fb, tb * P:(tb + 1) * P],
                                    wo_sb_res[:, fb, hb * HB:(hb + 1) * HB],
                                    start=(fb == 0),
                                    stop=(fb == NFO - 1))
                            nc.vector.tensor_copy(
                                po[:, hh * HB:(hh + 1) * HB], ps[:])
                        nc.scalar.dma_start(
                            out=partial_r[:, tb,
                                          half * HBH * HB:(half + 1) * HBH * HB],
                            in_=po[:])
                        yield
                emit_rs(g, tb_lo * P, tb_hi * P)

            # ---------------- driver --------------------------------------
            def chain_gens(*gens):
                for gen in gens:
                    if gen is not None:
                        yield from gen

            # bootstrap group 0's projection inline
            hd = emit_p1_head(0)
            for _ in emit_p1_chains(hd):
                pass
            prev_p3 = None
            for g in range(NG):
                q_sb = hd[6]
                att = attnring.tile([P, G, QS], BF16, tag="att", name="att")
                if g < NG - 1:
                    emit_attention(g, q_sb, att, 0, QS, filler=prev_p3)
                    if prev_p3 is not None:
                        for _ in prev_p3:   # drain remaining o_proj blocks
                            pass
                    prev_p3 = emit_p3(g, att, 0, KB_PER_QS)
                    hd = emit_p1_head(g + 1)
                    for _ in emit_p1_chains(hd):
                        pass
                else:
                    # last group: two q-halves so the first half's o_proj +
                    # RS overlap the second half's attention
                    emit_attention(g, q_sb, att, 0, QS // 2, filler=prev_p3)
                    if prev_p3 is not None:
                        for _ in prev_p3:
                            pass
                    pa = emit_p3(g, att, 0, KB_PER_QS // 2)
                    emit_attention(g, q_sb, att, QS // 2, QS, filler=pa)
                    for _ in pa:
                        pass
                    prev_p3 = emit_p3(g, att, KB_PER_QS // 2, KB_PER_QS)
            for _ in prev_p3:
                pass

    split_multi_waits(nc)
    return nc


_NC_CACHE = {}


def _get_nc(key):
    if key not in _NC_CACHE:
        _NC_CACHE[key] = build_nc(*key)
    return _NC_CACHE[key]


def run(hidden_states, w_qkv, w_o, cos, sin, B, S, HID, H, KV, D,
        trace=False):
    G = H // KV
    T = B * S
    GD = G * D
    QS = 512
    RPC = QS // NCORES
    assert KV == NCORES
    nc = _get_nc((B, S, HID, G, D))

    bf = ml_dtypes.bfloat16
    hiddenT = np.ascontiguousarray(
        hidden_states.reshape(T, HID).T).astype(bf)
    cosT = np.ascontiguousarray(
        cos.transpose(2, 0, 1).reshape(D, T)).astype(bf)
    sinT = np.ascontiguousarray(
        sin.transpose(2, 0, 1).reshape(D, T)).astype(np.float32)
    ssinT = sinT.copy()
    ssinT[:D // 2] *= -1.0
    ssinT = ssinT.astype(bf)

    in_maps = []
    for c in range(NCORES):
        qrows = w_qkv[c * GD:(c + 1) * GD]               # G query heads
        krows = w_qkv[H * D + c * D: H * D + (c + 1) * D]
        vrows = w_qkv[(H + KV) * D + c * D: (H + KV) * D + (c + 1) * D]
        w_c = np.concatenate([qrows, krows, vrows], axis=0)   # [F, HID]
        wqkvT = np.ascontiguousarray(w_c.T).astype(bf)
        woT = np.ascontiguousarray(w_o[:, c * GD:(c + 1) * GD].T).astype(bf)
        in_maps.append({
            "hiddenT": hiddenT, "wqkvT": wqkvT, "woT": woT,
            "cosT": cosT, "ssinT": ssinT,
        })

    res = run_bass_kernel_spmd(nc, in_maps, core_ids=list(range(NCORES)),
                               trace=trace)
    global LAST_RESULT
    LAST_RESULT = res

    # chunk list must match emit_p3/emit_rs: full groups of 512, the last
    # group split into two 256-token halves
    chunks = [(g * QS, QS) for g in range(T // QS - 1)]
    chunks += [(T - QS, QS // 2), (T - QS // 2, QS // 2)]
    full = np.empty((T, HID), dtype=np.float32)
    for c in range(NCORES):
        shard = res.results[c]["out"].astype(np.float32)  # [T//8, HID]
        o = 0
        for t0, ln in chunks:
            rpc = ln // NCORES
            full[t0 + c * rpc: t0 + (c + 1) * rpc] = shard[o:o + rpc]
            o += rpc
    return full.reshape(B, S, HID)


def kernel(hidden_states, w_qkv, w_o, cos, sin):
    return run(np.asarray(hidden_states), np.asarray(w_qkv), np.asarray(w_o),
               np.asarray(cos), np.asarray(sin),
               B_FULL, S_FULL, HID_FULL, H_FULL, KV_FULL, D_FULL,
               trace=bool(int(os.environ.get("KERNEL_TRACE", "0"))))
